# revision 40
# baseline (speedup 1.0000x reference)
"""DVAE GNN message-passing kernel for 8 Trainium2 NeuronCores.

Data parallel over batch B=2048 -> 256 graphs/core. Each core runs the full
20-step topological scan with all weights replicated.

Math (per sample b, step v in 0..19, Hfwd starts at 0):
  gated_u = sigmoid(Wg @ [H_u, e_u] + bg) * (Wm @ [H_u, e_u])
  Hin_v   = sum_u adj[b,u,v] * gated_u          (u >= v rows of Hfwd are 0,
            so gated_u there is a constant c_u)
  H_v     = GRUCell(x_v, Hin_v)
  mu,lv   = W1 @ H_19 + b1, W2 @ H_19 + b2

v2 design (scatter-forward, fp16):
- Message sums run scatter-forward: when G_u is produced, adj[b,u,v']*G_u is
  accumulated into per-target SBUF accumulators acc[v'] (fused stt MACs on
  DVE/GpSimd). Only the u -> u+1 update sits on the critical path, so the
  tensor engine stays dense and HAM-warm instead of idling behind the old
  per-step serial chain.
- acc[v'] is pre-initialized with the constant part sum_{u>=v'} adj*C_u via
  an upfront burst of k=20 matmuls (overlapped with the split weight DMA).
- Everything on the matmul path is fp16: weights, transposed activations,
  transpose identity. PE transposes run 1 cyc/row instead of fp32's 4,
  LDWEIGHTS can fast-load, DMA halves, and DVE tensor ops hit 2x/4x modes.
- Deferred scatter MACs are rate-limited per step (deadline-ordered) so the
  DVE load is even across steps; all 19 G vectors stay resident in SBUF.
- Matmuls are emitted grouped by stationary operand (activation chunk) so
  codegen can reuse LDWEIGHTS across the r/z/n (and zp/mp) matmuls.

Device layout: batch-major activations [128b, feat]; matmuls run with the
activation (transposed via PE) as the stationary operand and weights moving,
so outputs land batch-major in PSUM. Biases and the vertex-id one-hot
contributions are folded into the matmuls via ones-rows / one-hot k-chunks.
"""

import sys
import numpy as np

for _p in ("/opt/trn_rl_repo",):
    if _p not in sys.path:
        sys.path.insert(0, _p)

B, MAXN, NVT, HS, NZ = 2048, 20, 26, 501, 56
HS2 = HS + 1                  # 502
NVT_EFF = NVT + MAXN          # 46
XDIM = NVT_EFF + 1            # 47
NCORES = 8
BS = B // NCORES              # 256 samples per core
RZ = 2 * HS                   # 1002

# k-chunking of the augmented hidden axis (501 rows of H^T + ones row)
CH = [(0, 128), (128, 128), (256, 128), (384, 118)]  # covers 0..501 inclusive

# Deferred-scatter routing (job = one (u, target) pair = 2 tiles):
#   route 0: DVE fused stt (~0.73us/tile; TensorScalarPtr has no DVE fast mode)
#   route 2: GpSimd broadcast-mult + add (~1.2us/op, no fast modes)
# Each TARGET accumulator is affine to exactly one engine for its entire
# lifetime: cross-engine read-modify-write chains on one accumulator
# serialize through semaphores and head-of-line-block the engine FIFOs.
POOL_TARGETS = frozenset()   # empirically: cross-engine MACs lose to DVE
CAP_DVE_J = 10  # DVE-class deferred jobs per step
CAP_POOL_J = 3  # pool-class deferred jobs per step
N_EARLY = 3     # c2 jobs pulled forward to fill the pre-sigmoid DVE window


def _pack_layout():
    """Column layout (fp16 elements) of the single packed static tensor.

    Returns (entries, csplit, ncols); entries: name -> (row0, nrows, col0,
    ncols). Entries before csplit are needed by the upfront const-init burst
    and ship in the first of two DMAs.
    """
    ents = {}
    col = 0

    def put(name, row0, nrows, ncols):
        nonlocal col
        ents[name] = (row0, nrows, col, ncols)
        col += ncols

    put("pk", 0, 84, MAXN * BS)          # rows 0:48 X^T+ones, 64:84 adjT masked
    put("wxnc", 0, 84, HS2)              # rows 0:48 W_in^T+bias, 64:84 C
    put("ident", 0, 128, 128)
    put("adjg0", 0, 128, MAXN * MAXN)
    put("adjg1", 0, 128, MAXN * MAXN)
    csplit = col
    for i, (o, s) in enumerate(CH):
        put(f"wrzh{i}", 0, s, 2 * HS2)
    for i, (o, s) in enumerate(CH):
        put(f"whn{i}", 0, s, HS2)
    put("wrzx", 0, XDIM + 1, 2 * HS2)
    for i, (o, s) in enumerate(CH):
        put(f"wg{i}", 0, s, HS2)
    put("wgv", 0, MAXN, HS2)
    for i, (o, s) in enumerate(CH):
        put(f"wm{i}", 0, s, HS2)
    put("wmv", 0, MAXN, HS2)
    put("eye20", 0, MAXN, MAXN)
    for i, (o, s) in enumerate(CH):
        put(f"w12{i}", 0, s, 2 * NZ)
    # per-step diagonal matrices diag(adj[b, v-1, v]) for the critical
    # message MAC, run as a PE matmul accumulating into the transpose PSUM
    for v in range(1, MAXN):
        for t in range(2):
            put(f"diagc{v}_{t}", 0, 128, 128)
    return ents, csplit, col


def _scatter_schedule():
    """Static deadline-ordered schedule for deferred scatter MACs.

    Job (u, vp): acc[vp] += adj[:,u,vp] * G_u, available after step u,
    deadline before the transpose at step vp. The critical job (u, u+1) is
    always emitted at the top of step u+1; the rest are drained at up to
    CAP_DVE + CAP_GP jobs per step, nearest deadline first.

    Returns per-step lists: sched[v] = list of (u, vp, route) with route
    0=DVE stt, 2=pool mult+add, emitted mid-step v. The farthest-deadline
    N_POOL jobs go to pool, but never a target that a DVE job of the same
    step also touches.
    """
    import heapq
    hd, hp = [], []  # (deadline vp, u) heaps: DVE class, pool class
    sched = [[] for _ in range(MAXN)]
    for v in range(1, MAXN):
        u = v - 1
        for vp in range(v + 1, MAXN):
            heapq.heappush(hp if vp in POOL_TARGETS else hd, (vp, u))
        # mandatory: jobs whose deadline is the *next* step must go now
        for heap, cap, route in ((hd, CAP_DVE_J, 0), (hp, CAP_POOL_J, 2)):
            budget = cap
            while heap and (heap[0][0] <= v + 1 or budget > 0):
                vp, uu = heapq.heappop(heap)
                sched[v].append((uu, vp, route))
                budget -= 1
    assert not hd and not hp
    return sched


_PROG = None  # cached Bass program


def _build_program():
    import concourse.bass as bass  # noqa: F401
    import concourse.tile as tile
    from concourse import bacc, mybir

    f32 = mybir.dt.float32
    f16 = mybir.dt.float16
    AF = mybir.ActivationFunctionType
    OP = mybir.AluOpType

    nc = bacc.Bacc("TRN2", target_bir_lowering=False, debug=False)

    ents, csplit, ncols = _pack_layout()
    d_wpack = nc.dram_tensor("wpack", [128, ncols], f16, kind="ExternalInput").ap()
    d_out = nc.dram_tensor("out", [BS, 2 * NZ], f32, kind="ExternalOutput").ap()

    def mm(out, lhsT, rhs, start, stop):
        nc.tensor.matmul(out, lhsT, rhs, start=start, stop=stop)

    sched = _scatter_schedule()

    with tile.TileContext(nc) as tc:
        with (
            tc.tile_pool(name="statics", bufs=1) as sp,
            tc.tile_pool(name="accs", bufs=1) as accp,
            tc.tile_pool(name="gstore", bufs=1) as gp,
            tc.tile_pool(name="hint", bufs=2) as hip,
            tc.tile_pool(name="ht", bufs=2) as htp,
            tc.tile_pool(name="work", bufs=2) as wp,
            tc.tile_pool(name="ps", bufs=8, space="PSUM") as pp,
        ):
            WPACK = sp.tile([128, ncols], f16, tag="wpack", name="wpack")
            nc.sync.dma_start(WPACK[:, :csplit], d_wpack[:, :csplit])
            nc.sync.dma_start(WPACK[:, csplit:], d_wpack[:, csplit:])

            def sl(name):
                r0, nr, c0, ncl = ents[name]
                return WPACK[r0:r0 + nr, c0:c0 + ncl]

            PK = sl("pk")
            WRZH = [sl(f"wrzh{i}") for i in range(4)]
            WHN = [sl(f"whn{i}") for i in range(4)]
            WRZX = sl("wrzx")
            WXNC = sl("wxnc")
            WG = [sl(f"wg{i}") for i in range(4)]
            WM = [sl(f"wm{i}") for i in range(4)]
            WGV, WMV, EYE = sl("wgv"), sl("wmv"), sl("eye20")
            W12 = [sl(f"w12{i}") for i in range(4)]
            IDN = sl("ident")
            ADJG = [sl(f"adjg{t}") for t in range(2)]

            def adj_sc(t, u, vp):
                c = ents["adjg0"][2] if t == 0 else ents["adjg1"][2]
                c += u * MAXN + vp
                return WPACK[0:128, c:c + 1]

            DIAGC = {(v_, t_): sl(f"diagc{v_}_{t_}")
                     for v_ in range(1, MAXN) for t_ in range(2)}

            # ---- accumulators: acc[v][t] holds Hin_v as it builds up ----
            ACC = [[accp.tile([128, 512], f16, tag=f"acc{v}_{t}",
                              name=f"acc{v}_{t}") for t in range(2)]
                   for v in range(MAXN)]
            for v in range(MAXN):
                for t in range(2):
                    nc.gpsimd.memset(ACC[v][t][:, HS:HS + 1], 1.0)

            # ---- const init: acc[v] = sum_{u>=v} adj[b,u,v] * C[u] ----
            for v in range(MAXN):
                for t in range(2):
                    dps = pp.tile([128, 512], f32, tag="ps", name=f"cst{v}_{t}")
                    mm(dps[:, :HS2],
                       PK[64:84, v * BS + t * 128:v * BS + (t + 1) * 128],
                       WXNC[64:84, :], start=True, stop=True)
                    if (2 * v + t) % 2 == 0:
                        nc.scalar.copy(ACC[v][t][:, :HS], dps[:, :HS])
                    else:
                        nc.vector.tensor_copy(ACC[v][t][:, :HS], dps[:, :HS])

            # G storage: gated vectors per (vertex, batch-tile), all resident
            # (the ones column slot [HS] must stay 0: the critical diag-mm
            # reads G chunk 3 incl col HS and accumulates into the ones row)
            Gt = [[gp.tile([128, 512], f16, tag=f"g{_u}_{_t}",
                           name=f"g{_u}_{_t}")
                   for _t in range(2)] for _u in range(MAXN - 1)]
            for _u in range(MAXN - 1):
                for _t in range(2):
                    nc.gpsimd.memset(Gt[_u][_t][:, HS:HS + 1], 0.0)

            def hin_transposes(w_, HINT_w):
                """Build Hinaug^T for step w_: transpose acc[w_] as normal
                matmuls vs the identity; for w_>=1 the critical message MAC
                (G_{w_-1} scaled per-sample by adj[:,w_-1,w_]) accumulates
                into the same PSUM group as G_chunk^T @ diag(adj)."""
                for p in range(2):
                    tp = pp.tile([128, 512], f32, tag="ps",
                                 name=f"tpi{w_}_{p}")
                    for j in range(2):
                        i = 2 * p + j
                        o, w = CH[i]
                        for t in range(2):
                            dst = tp[:w, j * 256 + t * 128:
                                     j * 256 + (t + 1) * 128]
                            mm(dst, ACC[w_][t][:, o:o + w], IDN[:, :],
                               start=True, stop=(w_ == 0))
                            if w_ >= 1:
                                mm(dst, Gt[w_ - 1][t][:, o:o + w],
                                   DIAGC[(w_, t)], start=False, stop=True)
                    nc.scalar.copy(HINT_w[p][:, :], tp[:, :])

            # prelude: Hinaug^T for step 0
            HINT = [hip.tile([128, 512], f16, tag=f"hint{p}",
                             name=f"hint0_{p}") for p in range(2)]
            hin_transposes(0, HINT)

            def pool_mac(u, vp, t, tag):
                tmp = wp.tile([128, 512], f16, tag=tag, name=f"{tag}_{u}_{vp}")
                nc.gpsimd.tensor_tensor(
                    tmp[:, :HS], Gt[u][t][:, :HS],
                    adj_sc(t, u, vp).broadcast_to([128, HS]), OP.mult)
                nc.gpsimd.tensor_tensor(
                    ACC[vp][t][:, :HS], tmp[:, :HS],
                    ACC[vp][t][:, :HS], OP.add)

            HT_final = None
            for v in range(MAXN):
                def hsl(i, t):
                    return HINT[i // 2][0:CH[i][1], (i % 2) * 256 + t * 128:
                                        (i % 2) * 256 + (t + 1) * 128]

                # ---- GRU matmuls, grouped by stationary for LDW reuse ----
                rz0p, rz1p, hnp, inp = [], [], [], []
                for t in range(2):
                    xsl = PK[0:XDIM + 1, v * BS + t * 128:v * BS + (t + 1) * 128]
                    ps0 = pp.tile([128, 512], f32, tag="ps", name=f"rz0_{v}_{t}")
                    ps1 = pp.tile([128, 512], f32, tag="ps", name=f"rz1_{v}_{t}")
                    hn = pp.tile([128, 512], f32, tag="ps", name=f"hn{v}_{t}")
                    ip = pp.tile([128, 512], f32, tag="ps", name=f"in{v}_{t}")
                    for i in range(4):
                        h_ = hsl(i, t)
                        mm(ps0[:, :HS2], h_, WRZH[i][:, 0:HS2],
                           start=(i == 0), stop=False)
                        mm(ps1[:, :HS2], h_, WRZH[i][:, HS2:2 * HS2],
                           start=(i == 0), stop=False)
                        mm(hn[:, :HS2], h_, WHN[i][:, :],
                           start=(i == 0), stop=(i == 3))
                    mm(ps0[:, :HS2], xsl, WRZX[:, 0:HS2], start=False, stop=True)
                    mm(ps1[:, :HS2], xsl, WRZX[:, HS2:2 * HS2],
                       start=False, stop=True)
                    mm(ip[:, :HS2], xsl, WXNC[0:XDIM + 1, :],
                       start=True, stop=True)
                    rz0p.append(ps0)
                    rz1p.append(ps1)
                    hnp.append(hn)
                    inp.append(ip)

                # ---- scatter MACs due before acc[v+1]'s transposes ----
                for (u, vp, route) in sched[v]:
                    if vp != v + 1:
                        continue
                    for t in range(2):
                        if route == 2:
                            pool_mac(u, vp, t, f"pcd_{t}")
                        else:
                            nc.vector.scalar_tensor_tensor(
                                ACC[vp][t][:, :HS], Gt[u][t][:, :HS],
                                adj_sc(t, u, vp), ACC[vp][t][:, :HS],
                                OP.mult, OP.add)

                # ---- batch-major Hin for the blend: transpose HINT back
                # on the PE (PSUM f32 -> fp16); this carries the critical
                # message term the diag-mm already added, so no DVE stt ----
                hinb = []
                for t in range(2):
                    pb = pp.tile([128, 512], f32, tag="ps", name=f"pb{v}_{t}")
                    for i in range(4):
                        o, w = CH[i]
                        mm(pb[:, o:o + w], hsl(i, t), IDN[0:w, 0:w],
                           start=True, stop=True)
                    hb_ = wp.tile([128, 512], f16, tag=f"hinb{t}",
                                  name=f"hinb{v}_{t}")
                    nc.scalar.copy(hb_[:, :], pb[:, :])
                    hinb.append(hb_)

                # ---- early slack-rich scatters: fill DVE before r arrives --
                c2jobs = [(u_, vp_) for (u_, vp_, _r) in sched[v]
                          if vp_ != v + 1]
                for (u, vp) in c2jobs[:N_EARLY]:
                    for t in range(2):
                        nc.vector.scalar_tensor_tensor(
                            ACC[vp][t][:, :HS], Gt[u][t][:, :HS],
                            adj_sc(t, u, vp), ACC[vp][t][:, :HS],
                            OP.mult, OP.add)

                # ---- GRU elementwise ----
                hb = []
                for t in range(2):
                    r = wp.tile([128, 512], f16, tag=f"r{t}", name=f"r{v}_{t}")
                    nc.scalar.activation(r[:, :HS], rz0p[t][:, :HS], AF.Sigmoid)
                    z = wp.tile([128, 512], f16, tag=f"z{t}", name=f"z{v}_{t}")
                    nc.scalar.activation(z[:, :HS], rz1p[t][:, :HS], AF.Sigmoid)
                    # stage hn/in out of PSUM on the scalar engine so the
                    # DVE mult/add run in fp16 2x mode instead of 1x
                    hc = wp.tile([128, 512], f16, tag=f"hc{t}",
                                 name=f"hc{v}_{t}", bufs=1)
                    nc.scalar.copy(hc[:, :HS], hnp[t][:, :HS])
                    ic = wp.tile([128, 512], f16, tag=f"ic{t}",
                                 name=f"ic{v}_{t}", bufs=1)
                    nc.scalar.copy(ic[:, :HS], inp[t][:, :HS])
                    nc.vector.tensor_tensor(hc[:, :HS], r[:, :HS],
                                            hc[:, :HS], OP.mult)
                    nc.vector.tensor_tensor(hc[:, :HS], hc[:, :HS],
                                            ic[:, :HS], OP.add)
                    n = wp.tile([128, 512], f16, tag=f"n{t}", name=f"n{v}_{t}")
                    nc.scalar.activation(n[:, :HS], hc[:, :HS], AF.Tanh)
                    d = wp.tile([128, 512], f16, tag=f"d{t}", name=f"d{v}_{t}")
                    nc.vector.tensor_tensor(d[:, :HS], hinb[t][:, :HS],
                                            n[:, :HS], OP.subtract)
                    nc.vector.tensor_tensor(d[:, :HS], z[:, :HS],
                                            d[:, :HS], OP.mult)
                    h = wp.tile([128, 512], f16, tag=f"h{t}", name=f"h{v}_{t}")
                    nc.vector.tensor_tensor(h[:, :HS], d[:, :HS],
                                            n[:, :HS], OP.add)
                    if v < 2:
                        nc.gpsimd.memset(h[:, HS:HS + 1], 1.0)
                    hb.append(h)

                # ---- transpose H -> Haug^T chunk-pair tiles ----
                HT = [htp.tile([128, 512], f16, tag=f"ht{p}",
                               name=f"ht{v}_{p}") for p in range(2)]
                for p in range(2):
                    tp = pp.tile([128, 512], f16, tag="ps", name=f"tph{v}_{p}")
                    for j in range(2):
                        i = 2 * p + j
                        o, w = CH[i]
                        for t in range(2):
                            nc.tensor.transpose(
                                tp[:w, j * 256 + t * 128:j * 256 + (t + 1) * 128],
                                hb[t][:, o:o + w], IDN[:, :])
                    nc.scalar.copy(HT[p][:, :], tp[:, :])

                def htl(i, t):
                    return HT[i // 2][0:CH[i][1], (i % 2) * 256 + t * 128:
                                      (i % 2) * 256 + (t + 1) * 128]

                if v < MAXN - 1:
                    # ---- gated message for this vertex ----
                    # vid one-hot contribution: broadcast column of EYE
                    vsel = EYE[:, v:v + 1].broadcast_to([MAXN, 128])
                    for t in range(2):
                        zp = pp.tile([128, 512], f32, tag="ps", name=f"zp{v}_{t}")
                        mp = pp.tile([128, 512], f32, tag="ps", name=f"mp{v}_{t}")
                        for i in range(4):
                            h_ = htl(i, t)
                            mm(zp[:, :HS2], h_, WG[i][:, :],
                               start=(i == 0), stop=False)
                            mm(mp[:, :HS2], h_, WM[i][:, :],
                               start=(i == 0), stop=False)
                        mm(zp[:, :HS2], vsel, WGV[:, :], start=False, stop=True)
                        mm(mp[:, :HS2], vsel, WMV[:, :], start=False, stop=True)
                        sg = wp.tile([128, 512], f16, tag=f"sg{t}",
                                     name=f"sg{v}_{t}")
                        nc.scalar.activation(sg[:, :HS], zp[:, :HS], AF.Sigmoid)
                        mc = wp.tile([128, 512], f16, tag=f"mc{t}",
                                     name=f"mc{v}_{t}", bufs=1)
                        nc.scalar.copy(mc[:, :HS], mp[:, :HS])
                        nc.vector.tensor_tensor(Gt[v][t][:, :HS], sg[:, :HS],
                                                mc[:, :HS], OP.mult)
                if v == MAXN - 1:
                    HT_final = HT

                # ---- early: Hinaug^T for step v+1, overlapping gated mms ----
                if v + 1 < MAXN:
                    HINT = [hip.tile([128, 512], f16, tag=f"hint{p}",
                                     name=f"hint{v + 1}_{p}")
                            for p in range(2)]
                    hin_transposes(v + 1, HINT)

                # ---- remaining deferred scatter MACs (targets >= v+2) ----
                for (u, vp) in c2jobs[N_EARLY:]:
                    for t in range(2):
                        nc.vector.scalar_tensor_tensor(
                            ACC[vp][t][:, :HS], Gt[u][t][:, :HS],
                            adj_sc(t, u, vp), ACC[vp][t][:, :HS],
                            OP.mult, OP.add)

            # ---- readout ----
            for t in range(2):
                op = pp.tile([128, 512], f32, tag="ps", name=f"op{t}")
                for i in range(4):
                    ksl = HT_final[i // 2][0:CH[i][1], (i % 2) * 256 + t * 128:
                                           (i % 2) * 256 + (t + 1) * 128]
                    mm(op[:, :2 * NZ], ksl, W12[i][:, :],
                       start=(i == 0), stop=(i == 3))
                ob = wp.tile([128, 2 * NZ], f32, tag=f"ob{t}", name=f"ob{t}")
                nc.scalar.copy(ob[:, :], op[:, :2 * NZ])
                nc.sync.dma_start(d_out[t * 128:(t + 1) * 128, :], ob[:, :])

    nc.compile()
    return nc


def _host_prep(types, feats, adj, Wg, bg, Wm, W_ih, b_ih, W_hh, b_hh, W1, b1, W2, b2):
    """Build per-core input maps (numpy only)."""
    f = np.float32
    f16 = np.float16
    types = np.asarray(types).astype(np.int64)
    feats = np.asarray(feats, dtype=f)
    adj = np.asarray(adj, dtype=f)
    Wg, bg, Wm = np.asarray(Wg, f), np.asarray(bg, f), np.asarray(Wm, f)
    W_ih, b_ih = np.asarray(W_ih, f), np.asarray(b_ih, f)
    W_hh, b_hh = np.asarray(W_hh, f), np.asarray(b_hh, f)
    W1, b1 = np.asarray(W1, f), np.asarray(b1, f)
    W2, b2 = np.asarray(W2, f), np.asarray(b2, f)

    bsz = types.shape[0]
    bs = bsz // NCORES

    # X^T with ones row: [48, MAXN*bs] per core
    X = np.zeros((bsz, MAXN, XDIM + 1), dtype=f)
    onehot = np.eye(NVT_EFF, dtype=f)[types.reshape(-1) % NVT_EFF]
    X[:, :, :NVT_EFF] = onehot.reshape(bsz, MAXN, NVT_EFF)
    X[:, :, NVT_EFF] = feats
    X[:, :, XDIM] = 1.0

    # constant gated vectors c_u for zero hidden state
    zg = 1.0 / (1.0 + np.exp(-(bg[None, :] + Wg[:, HS:].T)))   # [20, 501]
    C = (zg * Wm[:, HS:].T).astype(f)

    def aug(wT, brow):
        return np.concatenate([wT, brow[None, :]], axis=0).astype(f)

    def pad_rz(a):          # [s, 1002] -> [s, 1004] with per-gate 502 halves
        o = np.zeros((a.shape[0], 2 * HS2), dtype=f)
        o[:, :HS] = a[:, :HS]
        o[:, HS2:HS2 + HS] = a[:, HS:]
        return o

    def pad_h(a):           # [s, 501] -> [s, 502]
        o = np.zeros((a.shape[0], HS2), dtype=f)
        o[:, :HS] = a
        return o

    wrzh = pad_rz(aug(W_hh[:RZ].T, b_hh[:RZ]))
    whn = pad_h(aug(W_hh[RZ:].T, b_hh[RZ:]))
    wrzx = pad_rz(aug(W_ih[:RZ].T, b_ih[:RZ]))
    wxnc = np.zeros((84, HS2), dtype=f)
    wxnc[:XDIM + 1] = pad_h(aug(W_ih[RZ:].T, b_ih[RZ:]))
    wxnc[64:84] = pad_h(C)
    wg = pad_h(np.concatenate([Wg[:, :HS].T, bg[None, :]], axis=0).astype(f))
    wgv = pad_h(np.ascontiguousarray(Wg[:, HS:].T))
    wm = pad_h(np.concatenate([Wm[:, :HS].T, np.zeros((1, HS), f)], axis=0))
    wmv = pad_h(np.ascontiguousarray(Wm[:, HS:].T))
    eye20 = np.eye(MAXN, dtype=f)
    w12 = np.concatenate([np.concatenate([W1.T, W2.T], axis=1),
                          np.concatenate([b1, b2])[None, :]], axis=0).astype(f)
    ident = np.eye(128, dtype=f)

    ents, _csplit, ncols = _pack_layout()

    def place(pack, name, arr):
        r0, nr, c0, ncl = ents[name]
        assert arr.shape == (nr, ncl), (name, arr.shape, (nr, ncl))
        pack[r0:r0 + nr, c0:c0 + ncl] = arr.astype(f16)

    umask = (np.arange(MAXN)[:, None] >= np.arange(MAXN)[None, :]).astype(f)

    in_maps = []
    for c in range(NCORES):
        slc = slice(c * bs, (c + 1) * bs)
        Xc = X[slc]                                   # [bs, 20, 48]
        xt = Xc.transpose(2, 1, 0).reshape(XDIM + 1, MAXN * bs)
        adjc = adj[slc]                               # [bs, 20, 20]
        # adjT[u, v*bs+b] = adj[b,u,v], zeroed where u < v (only u>=v used)
        adjm = adjc.transpose(1, 2, 0) * umask[:, :, None]
        pk = np.zeros((84, MAXN * bs), dtype=f)
        pk[:XDIM + 1] = xt
        pk[64:84] = adjm.reshape(MAXN, MAXN * bs)

        pack = np.zeros((128, ncols), dtype=f16)
        place(pack, "pk", pk)
        place(pack, "wxnc", wxnc)
        place(pack, "ident", ident)
        adjg = adjc.reshape(bs, MAXN * MAXN)
        place(pack, "adjg0", adjg[:128])
        place(pack, "adjg1", adjg[128:])
        for v in range(1, MAXN):
            for t in range(2):
                dg = np.zeros((128, 128), dtype=f)
                np.fill_diagonal(dg, adjc[t * 128:(t + 1) * 128, v - 1, v])
                place(pack, f"diagc{v}_{t}", dg)
        for i, (o, s) in enumerate(CH):
            place(pack, f"wrzh{i}", wrzh[o:o + s])
            place(pack, f"whn{i}", whn[o:o + s])
            place(pack, f"wg{i}", wg[o:o + s])
            place(pack, f"wm{i}", wm[o:o + s])
            place(pack, f"w12{i}", w12[o:o + s])
        place(pack, "wrzx", wrzx)
        place(pack, "wgv", wgv)
        place(pack, "wmv", wmv)
        place(pack, "eye20", eye20)
        in_maps.append(dict(wpack=pack))
    return in_maps


def _get_prog():
    global _PROG
    if _PROG is None:
        _PROG = _build_program()
    return _PROG


def kernel(**inputs):
    from concourse.bass_utils import run_bass_kernel_spmd
    nc = _get_prog()
    in_maps = _host_prep(**inputs)
    res = run_bass_kernel_spmd(nc, in_maps, core_ids=list(range(NCORES)))
    out = np.concatenate([r["out"] for r in res.results], axis=0)
    mu = np.ascontiguousarray(out[:, :NZ])
    logvar = np.ascontiguousarray(out[:, NZ:])
    return mu, logvar


# revision 41
# speedup vs baseline: 1.1314x; 1.1314x over previous
"""DVAE GNN message-passing kernel for 8 Trainium2 NeuronCores.

Data parallel over batch B=2048 -> 256 graphs/core. Each core runs the full
20-step topological scan with all weights replicated.

Math (per sample b, step v in 0..19, Hfwd starts at 0):
  gated_u = sigmoid(Wg @ [H_u, e_u] + bg) * (Wm @ [H_u, e_u])
  Hin_v   = sum_u adj[b,u,v] * gated_u          (u >= v rows of Hfwd are 0,
            so gated_u there is a constant c_u)
  H_v     = GRUCell(x_v, Hin_v)
  mu,lv   = W1 @ H_19 + b1, W2 @ H_19 + b2

v2 design (scatter-forward, fp16):
- Message sums run scatter-forward: when G_u is produced, adj[b,u,v']*G_u is
  accumulated into per-target SBUF accumulators acc[v'] (fused stt MACs on
  DVE/GpSimd). Only the u -> u+1 update sits on the critical path, so the
  tensor engine stays dense and HAM-warm instead of idling behind the old
  per-step serial chain.
- acc[v'] is pre-initialized with the constant part sum_{u>=v'} adj*C_u via
  an upfront burst of k=20 matmuls (overlapped with the split weight DMA).
- Everything on the matmul path is fp16: weights, transposed activations,
  transpose identity. PE transposes run 1 cyc/row instead of fp32's 4,
  LDWEIGHTS can fast-load, DMA halves, and DVE tensor ops hit 2x/4x modes.
- Deferred scatter MACs are rate-limited per step (deadline-ordered) so the
  DVE load is even across steps; all 19 G vectors stay resident in SBUF.
- Matmuls are emitted grouped by stationary operand (activation chunk) so
  codegen can reuse LDWEIGHTS across the r/z/n (and zp/mp) matmuls.

Device layout: batch-major activations [128b, feat]; matmuls run with the
activation (transposed via PE) as the stationary operand and weights moving,
so outputs land batch-major in PSUM. Biases and the vertex-id one-hot
contributions are folded into the matmuls via ones-rows / one-hot k-chunks.
"""

import sys
import numpy as np

for _p in ("/opt/trn_rl_repo",):
    if _p not in sys.path:
        sys.path.insert(0, _p)

B, MAXN, NVT, HS, NZ = 2048, 20, 26, 501, 56
HS2 = HS + 1                  # 502
NVT_EFF = NVT + MAXN          # 46
XDIM = NVT_EFF + 1            # 47
NCORES = 8
BS = B // NCORES              # 256 samples per core
RZ = 2 * HS                   # 1002

# k-chunking of the augmented hidden axis (501 rows of H^T + ones row)
CH = [(0, 128), (128, 128), (256, 128), (384, 118)]  # covers 0..501 inclusive

# Deferred-scatter routing (job = one (u, target) pair = 2 tiles):
#   route 0: DVE fused stt (~0.73us/tile; TensorScalarPtr has no DVE fast mode)
#   route 2: GpSimd broadcast-mult + add (~1.2us/op, no fast modes)
# Each TARGET accumulator is affine to exactly one engine for its entire
# lifetime: cross-engine read-modify-write chains on one accumulator
# serialize through semaphores and head-of-line-block the engine FIFOs.
POOL_TARGETS = frozenset()   # empirically: cross-engine MACs lose to DVE
CAP_DVE_J = 10  # DVE-class deferred jobs per step
CAP_POOL_J = 3  # pool-class deferred jobs per step
N_EARLY = 3     # c2 jobs pulled forward to fill the pre-sigmoid DVE window


def _pack_layout():
    """Column layout (fp16 elements) of the single packed static tensor.

    Returns (entries, csplit, ncols); entries: name -> (row0, nrows, col0,
    ncols). Entries before csplit are needed by the upfront const-init burst
    and ship in the first of two DMAs.
    """
    ents = {}
    col = 0

    def put(name, row0, nrows, ncols):
        nonlocal col
        ents[name] = (row0, nrows, col, ncols)
        col += ncols

    put("pk", 0, 84, MAXN * BS)          # rows 0:48 X^T+ones, 64:84 adjT masked
    put("wxnc", 0, 84, HS2)              # rows 0:48 W_in^T+bias, 64:84 C
    put("ident", 0, 128, 128)
    put("adjg0", 0, 128, MAXN * MAXN)
    put("adjg1", 0, 128, MAXN * MAXN)
    csplit = col
    for i, (o, s) in enumerate(CH):
        put(f"wrzh{i}", 0, s, 2 * HS2)
    for i, (o, s) in enumerate(CH):
        put(f"whn{i}", 0, s, HS2)
    put("wrzx", 0, XDIM + 1, 2 * HS2)
    for i, (o, s) in enumerate(CH):
        put(f"wg{i}", 0, s, HS2)
    put("wgv", 0, MAXN, HS2)
    for i, (o, s) in enumerate(CH):
        put(f"wm{i}", 0, s, HS2)
    put("wmv", 0, MAXN, HS2)
    put("eye20", 0, MAXN, MAXN)
    for i, (o, s) in enumerate(CH):
        put(f"w12{i}", 0, s, 2 * NZ)
    # per-step diagonal matrices diag(adj[b, v-1, v]) for the critical
    # message MAC, run as a PE matmul accumulating into the transpose PSUM
    for v in range(1, MAXN):
        for t in range(2):
            put(f"diagc{v}_{t}", 0, 128, 128)
    return ents, csplit, col


def _scatter_schedule():
    """Static deadline-ordered schedule for deferred scatter MACs.

    Job (u, vp): acc[vp] += adj[:,u,vp] * G_u, available after step u,
    deadline before the transpose at step vp. The critical job (u, u+1) is
    always emitted at the top of step u+1; the rest are drained at up to
    CAP_DVE + CAP_GP jobs per step, nearest deadline first.

    Returns per-step lists: sched[v] = list of (u, vp, route) with route
    0=DVE stt, 2=pool mult+add, emitted mid-step v. The farthest-deadline
    N_POOL jobs go to pool, but never a target that a DVE job of the same
    step also touches.
    """
    import heapq
    hd, hp = [], []  # (deadline vp, u) heaps: DVE class, pool class
    sched = [[] for _ in range(MAXN)]
    for v in range(1, MAXN):
        u = v - 1
        for vp in range(v + 1, MAXN):
            heapq.heappush(hp if vp in POOL_TARGETS else hd, (vp, u))
        # mandatory: jobs whose deadline is the *next* step must go now
        for heap, cap, route in ((hd, CAP_DVE_J, 0), (hp, CAP_POOL_J, 2)):
            budget = cap
            while heap and (heap[0][0] <= v + 1 or budget > 0):
                vp, uu = heapq.heappop(heap)
                sched[v].append((uu, vp, route))
                budget -= 1
    assert not hd and not hp
    return sched


_PROG = None  # cached Bass program


def _build_program():
    import concourse.bass as bass  # noqa: F401
    import concourse.tile as tile
    from concourse import bacc, mybir

    f32 = mybir.dt.float32
    f16 = mybir.dt.float16
    AF = mybir.ActivationFunctionType
    OP = mybir.AluOpType

    nc = bacc.Bacc("TRN2", target_bir_lowering=False, debug=False)

    ents, csplit, ncols = _pack_layout()
    d_wpack = nc.dram_tensor("wpack", [128, ncols], f16, kind="ExternalInput").ap()
    d_out = nc.dram_tensor("out", [BS, 2 * NZ], f32, kind="ExternalOutput").ap()

    def mm(out, lhsT, rhs, start, stop):
        nc.tensor.matmul(out, lhsT, rhs, start=start, stop=stop)

    sched = _scatter_schedule()

    with tile.TileContext(nc) as tc:
        with (
            tc.tile_pool(name="statics", bufs=1) as sp,
            tc.tile_pool(name="accs", bufs=1) as accp,
            tc.tile_pool(name="gstore", bufs=1) as gp,
            tc.tile_pool(name="hint", bufs=2) as hip,
            tc.tile_pool(name="ht", bufs=2) as htp,
            tc.tile_pool(name="work", bufs=2) as wp,
            tc.tile_pool(name="ps", bufs=8, space="PSUM") as pp,
        ):
            WPACK = sp.tile([128, ncols], f16, tag="wpack", name="wpack")
            nc.sync.dma_start(WPACK[:, :csplit], d_wpack[:, :csplit])
            nc.sync.dma_start(WPACK[:, csplit:], d_wpack[:, csplit:])

            def sl(name):
                r0, nr, c0, ncl = ents[name]
                return WPACK[r0:r0 + nr, c0:c0 + ncl]

            PK = sl("pk")
            WRZH = [sl(f"wrzh{i}") for i in range(4)]
            WHN = [sl(f"whn{i}") for i in range(4)]
            WRZX = sl("wrzx")
            WXNC = sl("wxnc")
            WG = [sl(f"wg{i}") for i in range(4)]
            WM = [sl(f"wm{i}") for i in range(4)]
            WGV, WMV, EYE = sl("wgv"), sl("wmv"), sl("eye20")
            W12 = [sl(f"w12{i}") for i in range(4)]
            IDN = sl("ident")
            ADJG = [sl(f"adjg{t}") for t in range(2)]

            def adj_sc(t, u, vp):
                c = ents["adjg0"][2] if t == 0 else ents["adjg1"][2]
                c += u * MAXN + vp
                return WPACK[0:128, c:c + 1]

            DIAGC = {(v_, t_): sl(f"diagc{v_}_{t_}")
                     for v_ in range(1, MAXN) for t_ in range(2)}

            # ---- accumulators: acc[v][t] holds Hin_v as it builds up ----
            ACC = [[accp.tile([128, 512], f16, tag=f"acc{v}_{t}",
                              name=f"acc{v}_{t}") for t in range(2)]
                   for v in range(MAXN)]
            for v in range(MAXN):
                for t in range(2):
                    nc.gpsimd.memset(ACC[v][t][:, HS:HS + 1], 1.0)

            # ---- const init: acc[v] = sum_{u>=v} adj[b,u,v] * C[u] ----
            for v in range(MAXN):
                for t in range(2):
                    dps = pp.tile([128, 512], f32, tag="ps", name=f"cst{v}_{t}")
                    mm(dps[:, :HS2],
                       PK[64:84, v * BS + t * 128:v * BS + (t + 1) * 128],
                       WXNC[64:84, :], start=True, stop=True)
                    if (2 * v + t) % 2 == 0:
                        nc.scalar.copy(ACC[v][t][:, :HS], dps[:, :HS])
                    else:
                        nc.vector.tensor_copy(ACC[v][t][:, :HS], dps[:, :HS])

            # G storage: gated vectors per (vertex, batch-tile), all resident
            # (the ones column slot [HS] must stay 0: the critical diag-mm
            # reads G chunk 3 incl col HS and accumulates into the ones row)
            Gt = [[gp.tile([128, 512], f16, tag=f"g{_u}_{_t}",
                           name=f"g{_u}_{_t}")
                   for _t in range(2)] for _u in range(MAXN - 1)]
            for _u in range(MAXN - 1):
                for _t in range(2):
                    nc.gpsimd.memset(Gt[_u][_t][:, HS:HS + 1], 0.0)

            def hin_transposes(w_, HINT_w):
                """Build Hinaug^T for step w_: transpose acc[w_] as normal
                matmuls vs the identity; for w_>=1 the critical message MAC
                (G_{w_-1} scaled per-sample by adj[:,w_-1,w_]) accumulates
                into the same PSUM group as G_chunk^T @ diag(adj)."""
                for p in range(2):
                    tp = pp.tile([128, 512], f32, tag="ps",
                                 name=f"tpi{w_}_{p}")
                    for j in range(2):
                        i = 2 * p + j
                        o, w = CH[i]
                        for t in range(2):
                            dst = tp[:w, j * 256 + t * 128:
                                     j * 256 + (t + 1) * 128]
                            mm(dst, ACC[w_][t][:, o:o + w], IDN[:, :],
                               start=True, stop=(w_ == 0))
                            if w_ >= 1:
                                mm(dst, Gt[w_ - 1][t][:, o:o + w],
                                   DIAGC[(w_, t)], start=False, stop=True)
                    nc.scalar.copy(HINT_w[p][:, :], tp[:, :])

            # prelude: Hinaug^T for step 0
            HINT = [hip.tile([128, 512], f16, tag=f"hint{p}",
                             name=f"hint0_{p}") for p in range(2)]
            hin_transposes(0, HINT)

            def pool_mac(u, vp, t, tag):
                tmp = wp.tile([128, 512], f16, tag=tag, name=f"{tag}_{u}_{vp}")
                nc.gpsimd.tensor_tensor(
                    tmp[:, :HS], Gt[u][t][:, :HS],
                    adj_sc(t, u, vp).broadcast_to([128, HS]), OP.mult)
                nc.gpsimd.tensor_tensor(
                    ACC[vp][t][:, :HS], tmp[:, :HS],
                    ACC[vp][t][:, :HS], OP.add)

            HT_final = None
            for v in range(MAXN):
                def hsl(i, t):
                    return HINT[i // 2][0:CH[i][1], (i % 2) * 256 + t * 128:
                                        (i % 2) * 256 + (t + 1) * 128]

                # ---- GRU matmuls, grouped by stationary for LDW reuse ----
                rz0p, rz1p, hnp, inp = [], [], [], []
                for t in range(2):
                    xsl = PK[0:XDIM + 1, v * BS + t * 128:v * BS + (t + 1) * 128]
                    ps0 = pp.tile([128, 512], f32, tag="ps", name=f"rz0_{v}_{t}")
                    ps1 = pp.tile([128, 512], f32, tag="ps", name=f"rz1_{v}_{t}")
                    hn = pp.tile([128, 512], f32, tag="ps", name=f"hn{v}_{t}")
                    ip = pp.tile([128, 512], f32, tag="ps", name=f"in{v}_{t}")
                    for i in range(4):
                        h_ = hsl(i, t)
                        mm(ps0[:, :HS2], h_, WRZH[i][:, 0:HS2],
                           start=(i == 0), stop=False)
                        mm(ps1[:, :HS2], h_, WRZH[i][:, HS2:2 * HS2],
                           start=(i == 0), stop=False)
                        mm(hn[:, :HS2], h_, WHN[i][:, :],
                           start=(i == 0), stop=(i == 3))
                    mm(ps0[:, :HS2], xsl, WRZX[:, 0:HS2], start=False, stop=True)
                    mm(ps1[:, :HS2], xsl, WRZX[:, HS2:2 * HS2],
                       start=False, stop=True)
                    mm(ip[:, :HS2], xsl, WXNC[0:XDIM + 1, :],
                       start=True, stop=True)
                    rz0p.append(ps0)
                    rz1p.append(ps1)
                    hnp.append(hn)
                    inp.append(ip)

                # ---- scatter MACs due before acc[v+1]'s transposes ----
                for (u, vp, route) in sched[v]:
                    if vp != v + 1:
                        continue
                    for t in range(2):
                        if route == 2:
                            pool_mac(u, vp, t, f"pcd_{t}")
                        else:
                            nc.vector.scalar_tensor_tensor(
                                ACC[vp][t][:, :HS], Gt[u][t][:, :HS],
                                adj_sc(t, u, vp), ACC[vp][t][:, :HS],
                                OP.mult, OP.add)

                # ---- batch-major Hin for the blend: transpose HINT back
                # on the PE (PSUM f32 -> fp16); this carries the critical
                # message term the diag-mm already added, so no DVE stt ----
                hinb = []
                for t in range(2):
                    pb = pp.tile([128, 512], f32, tag="ps", name=f"pb{v}_{t}")
                    for i in range(4):
                        o, w = CH[i]
                        mm(pb[:, o:o + w], hsl(i, t), IDN[0:w, 0:w],
                           start=True, stop=True)
                    hb_ = wp.tile([128, 512], f16, tag=f"hinb{t}",
                                  name=f"hinb{v}_{t}")
                    nc.scalar.copy(hb_[:, :], pb[:, :])
                    hinb.append(hb_)

                # ---- early slack-rich scatters: fill DVE before r arrives --
                c2jobs = [(u_, vp_) for (u_, vp_, _r) in sched[v]
                          if vp_ != v + 1]
                for (u, vp) in c2jobs[:N_EARLY]:
                    for t in range(2):
                        nc.vector.scalar_tensor_tensor(
                            ACC[vp][t][:, :HS], Gt[u][t][:, :HS],
                            adj_sc(t, u, vp), ACC[vp][t][:, :HS],
                            OP.mult, OP.add)

                # ---- GRU elementwise ----
                hb = []
                for t in range(2):
                    r = wp.tile([128, 512], f16, tag=f"r{t}", name=f"r{v}_{t}")
                    nc.scalar.activation(r[:, :HS], rz0p[t][:, :HS], AF.Sigmoid)
                    z = wp.tile([128, 512], f16, tag=f"z{t}", name=f"z{v}_{t}")
                    nc.scalar.activation(z[:, :HS], rz1p[t][:, :HS], AF.Sigmoid)
                    q = wp.tile([128, 512], f16, tag=f"q{t}", name=f"q{v}_{t}")
                    nc.vector.tensor_tensor(q[:, :HS], r[:, :HS],
                                            hnp[t][:, :HS], OP.mult)
                    nc.vector.tensor_tensor(q[:, :HS], q[:, :HS],
                                            inp[t][:, :HS], OP.add)
                    n = wp.tile([128, 512], f16, tag=f"n{t}", name=f"n{v}_{t}")
                    nc.scalar.activation(n[:, :HS], q[:, :HS], AF.Tanh)
                    d = wp.tile([128, 512], f16, tag=f"d{t}", name=f"d{v}_{t}")
                    nc.vector.tensor_tensor(d[:, :HS], hinb[t][:, :HS],
                                            n[:, :HS], OP.subtract)
                    nc.vector.tensor_tensor(d[:, :HS], z[:, :HS],
                                            d[:, :HS], OP.mult)
                    h = wp.tile([128, 512], f16, tag=f"h{t}", name=f"h{v}_{t}")
                    nc.vector.tensor_tensor(h[:, :HS], d[:, :HS],
                                            n[:, :HS], OP.add)
                    if v < 2:
                        nc.gpsimd.memset(h[:, HS:HS + 1], 1.0)
                    hb.append(h)

                # ---- transpose H -> Haug^T chunk-pair tiles ----
                HT = [htp.tile([128, 512], f16, tag=f"ht{p}",
                               name=f"ht{v}_{p}") for p in range(2)]
                for p in range(2):
                    tp = pp.tile([128, 512], f16, tag="ps", name=f"tph{v}_{p}")
                    for j in range(2):
                        i = 2 * p + j
                        o, w = CH[i]
                        for t in range(2):
                            nc.tensor.transpose(
                                tp[:w, j * 256 + t * 128:j * 256 + (t + 1) * 128],
                                hb[t][:, o:o + w], IDN[:, :])
                    nc.scalar.copy(HT[p][:, :], tp[:, :])

                def htl(i, t):
                    return HT[i // 2][0:CH[i][1], (i % 2) * 256 + t * 128:
                                      (i % 2) * 256 + (t + 1) * 128]

                if v < MAXN - 1:
                    # ---- gated message for this vertex ----
                    # vid one-hot contribution: broadcast column of EYE
                    vsel = EYE[:, v:v + 1].broadcast_to([MAXN, 128])
                    for t in range(2):
                        zp = pp.tile([128, 512], f32, tag="ps", name=f"zp{v}_{t}")
                        mp = pp.tile([128, 512], f32, tag="ps", name=f"mp{v}_{t}")
                        for i in range(4):
                            h_ = htl(i, t)
                            mm(zp[:, :HS2], h_, WG[i][:, :],
                               start=(i == 0), stop=False)
                            mm(mp[:, :HS2], h_, WM[i][:, :],
                               start=(i == 0), stop=False)
                        mm(zp[:, :HS2], vsel, WGV[:, :], start=False, stop=True)
                        mm(mp[:, :HS2], vsel, WMV[:, :], start=False, stop=True)
                        sg = wp.tile([128, 512], f16, tag=f"sg{t}",
                                     name=f"sg{v}_{t}")
                        nc.scalar.activation(sg[:, :HS], zp[:, :HS], AF.Sigmoid)
                        nc.vector.tensor_tensor(Gt[v][t][:, :HS], sg[:, :HS],
                                                mp[:, :HS], OP.mult)
                if v == MAXN - 1:
                    HT_final = HT

                # ---- early: Hinaug^T for step v+1, overlapping gated mms ----
                if v + 1 < MAXN:
                    HINT = [hip.tile([128, 512], f16, tag=f"hint{p}",
                                     name=f"hint{v + 1}_{p}")
                            for p in range(2)]
                    hin_transposes(v + 1, HINT)

                # ---- remaining deferred scatter MACs (targets >= v+2) ----
                for (u, vp) in c2jobs[N_EARLY:]:
                    for t in range(2):
                        nc.vector.scalar_tensor_tensor(
                            ACC[vp][t][:, :HS], Gt[u][t][:, :HS],
                            adj_sc(t, u, vp), ACC[vp][t][:, :HS],
                            OP.mult, OP.add)

            # ---- readout ----
            for t in range(2):
                op = pp.tile([128, 512], f32, tag="ps", name=f"op{t}")
                for i in range(4):
                    ksl = HT_final[i // 2][0:CH[i][1], (i % 2) * 256 + t * 128:
                                           (i % 2) * 256 + (t + 1) * 128]
                    mm(op[:, :2 * NZ], ksl, W12[i][:, :],
                       start=(i == 0), stop=(i == 3))
                ob = wp.tile([128, 2 * NZ], f32, tag=f"ob{t}", name=f"ob{t}")
                nc.scalar.copy(ob[:, :], op[:, :2 * NZ])
                nc.sync.dma_start(d_out[t * 128:(t + 1) * 128, :], ob[:, :])

    nc.compile()
    return nc


def _host_prep(types, feats, adj, Wg, bg, Wm, W_ih, b_ih, W_hh, b_hh, W1, b1, W2, b2):
    """Build per-core input maps (numpy only)."""
    f = np.float32
    f16 = np.float16
    types = np.asarray(types).astype(np.int64)
    feats = np.asarray(feats, dtype=f)
    adj = np.asarray(adj, dtype=f)
    Wg, bg, Wm = np.asarray(Wg, f), np.asarray(bg, f), np.asarray(Wm, f)
    W_ih, b_ih = np.asarray(W_ih, f), np.asarray(b_ih, f)
    W_hh, b_hh = np.asarray(W_hh, f), np.asarray(b_hh, f)
    W1, b1 = np.asarray(W1, f), np.asarray(b1, f)
    W2, b2 = np.asarray(W2, f), np.asarray(b2, f)

    bsz = types.shape[0]
    bs = bsz // NCORES

    # X^T with ones row: [48, MAXN*bs] per core
    X = np.zeros((bsz, MAXN, XDIM + 1), dtype=f)
    onehot = np.eye(NVT_EFF, dtype=f)[types.reshape(-1) % NVT_EFF]
    X[:, :, :NVT_EFF] = onehot.reshape(bsz, MAXN, NVT_EFF)
    X[:, :, NVT_EFF] = feats
    X[:, :, XDIM] = 1.0

    # constant gated vectors c_u for zero hidden state
    zg = 1.0 / (1.0 + np.exp(-(bg[None, :] + Wg[:, HS:].T)))   # [20, 501]
    C = (zg * Wm[:, HS:].T).astype(f)

    def aug(wT, brow):
        return np.concatenate([wT, brow[None, :]], axis=0).astype(f)

    def pad_rz(a):          # [s, 1002] -> [s, 1004] with per-gate 502 halves
        o = np.zeros((a.shape[0], 2 * HS2), dtype=f)
        o[:, :HS] = a[:, :HS]
        o[:, HS2:HS2 + HS] = a[:, HS:]
        return o

    def pad_h(a):           # [s, 501] -> [s, 502]
        o = np.zeros((a.shape[0], HS2), dtype=f)
        o[:, :HS] = a
        return o

    wrzh = pad_rz(aug(W_hh[:RZ].T, b_hh[:RZ]))
    whn = pad_h(aug(W_hh[RZ:].T, b_hh[RZ:]))
    wrzx = pad_rz(aug(W_ih[:RZ].T, b_ih[:RZ]))
    wxnc = np.zeros((84, HS2), dtype=f)
    wxnc[:XDIM + 1] = pad_h(aug(W_ih[RZ:].T, b_ih[RZ:]))
    wxnc[64:84] = pad_h(C)
    wg = pad_h(np.concatenate([Wg[:, :HS].T, bg[None, :]], axis=0).astype(f))
    wgv = pad_h(np.ascontiguousarray(Wg[:, HS:].T))
    wm = pad_h(np.concatenate([Wm[:, :HS].T, np.zeros((1, HS), f)], axis=0))
    wmv = pad_h(np.ascontiguousarray(Wm[:, HS:].T))
    eye20 = np.eye(MAXN, dtype=f)
    w12 = np.concatenate([np.concatenate([W1.T, W2.T], axis=1),
                          np.concatenate([b1, b2])[None, :]], axis=0).astype(f)
    ident = np.eye(128, dtype=f)

    ents, _csplit, ncols = _pack_layout()

    def place(pack, name, arr):
        r0, nr, c0, ncl = ents[name]
        assert arr.shape == (nr, ncl), (name, arr.shape, (nr, ncl))
        pack[r0:r0 + nr, c0:c0 + ncl] = arr.astype(f16)

    umask = (np.arange(MAXN)[:, None] >= np.arange(MAXN)[None, :]).astype(f)

    in_maps = []
    for c in range(NCORES):
        slc = slice(c * bs, (c + 1) * bs)
        Xc = X[slc]                                   # [bs, 20, 48]
        xt = Xc.transpose(2, 1, 0).reshape(XDIM + 1, MAXN * bs)
        adjc = adj[slc]                               # [bs, 20, 20]
        # adjT[u, v*bs+b] = adj[b,u,v], zeroed where u < v (only u>=v used)
        adjm = adjc.transpose(1, 2, 0) * umask[:, :, None]
        pk = np.zeros((84, MAXN * bs), dtype=f)
        pk[:XDIM + 1] = xt
        pk[64:84] = adjm.reshape(MAXN, MAXN * bs)

        pack = np.zeros((128, ncols), dtype=f16)
        place(pack, "pk", pk)
        place(pack, "wxnc", wxnc)
        place(pack, "ident", ident)
        adjg = adjc.reshape(bs, MAXN * MAXN)
        place(pack, "adjg0", adjg[:128])
        place(pack, "adjg1", adjg[128:])
        for v in range(1, MAXN):
            for t in range(2):
                dg = np.zeros((128, 128), dtype=f)
                np.fill_diagonal(dg, adjc[t * 128:(t + 1) * 128, v - 1, v])
                place(pack, f"diagc{v}_{t}", dg)
        for i, (o, s) in enumerate(CH):
            place(pack, f"wrzh{i}", wrzh[o:o + s])
            place(pack, f"whn{i}", whn[o:o + s])
            place(pack, f"wg{i}", wg[o:o + s])
            place(pack, f"wm{i}", wm[o:o + s])
            place(pack, f"w12{i}", w12[o:o + s])
        place(pack, "wrzx", wrzx)
        place(pack, "wgv", wgv)
        place(pack, "wmv", wmv)
        place(pack, "eye20", eye20)
        in_maps.append(dict(wpack=pack))
    return in_maps


def _get_prog():
    global _PROG
    if _PROG is None:
        _PROG = _build_program()
    return _PROG


def kernel(**inputs):
    from concourse.bass_utils import run_bass_kernel_spmd
    nc = _get_prog()
    in_maps = _host_prep(**inputs)
    res = run_bass_kernel_spmd(nc, in_maps, core_ids=list(range(NCORES)))
    out = np.concatenate([r["out"] for r in res.results], axis=0)
    mu = np.ascontiguousarray(out[:, :NZ])
    logvar = np.ascontiguousarray(out[:, NZ:])
    return mu, logvar
